# revision 18
# baseline (speedup 1.0000x reference)
"""Trainium2 Bass kernel for nn_ExplicitGCN (GCN message passing).

Strategy: data-parallel over batch B=8 (one batch per NeuronCore). All per-layer
tensors stay resident in SBUF. Per GCN layer:
  - h = x @ W^T on the TensorEngine (features-on-partitions "transposed" layout
    for the input, token layout [v%128, v//128, f] for the output in PSUM),
  - pre-scale by dinv (GCN symmetric norm, folded to per-vertex scaling),
  - neighbor aggregation: self-loop via a strided copy + K rounds of
    dma_scatter_add (SBUF parity destination, CCE accumulate) where round k
    scatters the k-th out-edge of every vertex (vertices pre-sorted by
    out-degree on the host so each round's sources are a dense slot prefix),
  - transpose back to [f, v] layout fused with the dinv post-scale via a
    matmul against per-block diag(dinv) stationaries, relu+bias at PSUM
    evacuation.
Host side does index preprocessing only (degree, sort, relabel, round tables)
plus the tiny latent @ W_lat fold; all O(V*H) / O(E*H) math runs on device.
"""

import sys
import types

sys.path.insert(0, '/opt/trn_rl_repo')
sys.path.insert(0, '/root/.axon_site')

import numpy as np
import ml_dtypes

BF16 = ml_dtypes.bfloat16

V = 10000
E = 60000
B = 8
HID = 256
LAT = 512
NL = 5
NB = 79                 # vertex blocks of 128
VP = NB * 128           # padded vertex count 10112
FB = HID // 128         # feature blocks = 2
MM_CHUNK = 512


def _install_axon_ntff_shim():
    """Provide antenv.axon_hooks so run_bass_kernel_spmd(trace=True) works."""
    try:
        import antenv
        if 'antenv.axon_hooks' in sys.modules:
            return
        from trn_agent_boot.trn_boot import _ntff_profile_via_ctypes
        hook = _ntff_profile_via_ctypes('/opt/axon/libaxon_pjrt.so')
        mod = types.ModuleType('antenv.axon_hooks')
        mod.get_axon_ntff_profile_hook = lambda: hook
        mod.set_axon_ntff_profile_hook = lambda h: None
        sys.modules['antenv.axon_hooks'] = mod
        antenv.axon_hooks = mod
    except Exception:
        pass


def preprocess(inputs):
    """Host-side index/layout preprocessing. Returns (device_inputs, meta)."""
    vertex_xyz = np.asarray(inputs['vertex_xyz'], dtype=np.float32)
    latent = np.asarray(inputs['latent'], dtype=np.float32)
    edge_index = np.asarray(inputs['edge_index']).astype(np.int64)
    input_W = np.asarray(inputs['input_W'], dtype=np.float32)
    input_b = np.asarray(inputs['input_b'], dtype=np.float32)
    conv_W = np.asarray(inputs['conv_W'], dtype=np.float32)
    conv_b = np.asarray(inputs['conv_b'], dtype=np.float32)
    out_W = np.asarray(inputs['out_W'], dtype=np.float32)
    out_b = np.asarray(inputs['out_b'], dtype=np.float32)

    src, dst = edge_index[0], edge_index[1]
    od = np.bincount(src, minlength=V)                    # out-degree, no self loop
    indeg = np.bincount(dst, minlength=V).astype(np.float32) + 1.0
    dinv = 1.0 / np.sqrt(indeg)

    order = np.argsort(-od, kind='stable')                # vertices by out-degree desc
    rankv = np.empty(V, dtype=np.int64)
    rankv[order] = np.arange(V)
    srcr = rankv[src]
    dstr = rankv[dst]
    odr = od[order]
    maxod = int(odr[0]) if V else 0

    eo = np.argsort(srcr, kind='stable')
    ds = dstr[eo].astype(np.int64)
    start = np.searchsorted(srcr[eo], np.arange(V + 1))

    # Assign each vertex's out-edges to rounds 0..od-1 such that within a
    # round all destinations are DISTINCT (the SDMA CCE accumulate is not
    # atomic across engines; duplicate dsts in one call lose updates).
    # Per-vertex bipartite matching (edge -> round), greedy + augmenting.
    nks = [int((odr > k).sum()) for k in range(maxod)]
    used = np.zeros((maxod, V), dtype=bool)
    round_dsts = [np.full(nks[k], -1, dtype=np.int64) for k in range(maxod)]
    overflow = []
    for v in reversed(range(V)):   # low out-degree (most constrained) first
        dlist = ds[start[v]:start[v + 1]]
        m = len(dlist)
        if m == 0:
            continue
        col_edge = {}

        def try_assign(i, seen):
            for k in range(m):
                if k in seen or used[k, dlist[i]]:
                    continue
                seen.add(k)
                j = col_edge.get(k)
                if j is None or try_assign(j, seen):
                    col_edge[k] = i
                    return True
            return False

        order_e = sorted(range(m),
                         key=lambda i: -int(used[:m, dlist[i]].sum()))
        for i in order_e:
            if not try_assign(i, set()):
                overflow.append((v, int(dlist[i])))
        for k, i in col_edge.items():
            used[k, dlist[i]] = True
            round_dsts[k][v] = dlist[i]

    # overflow edges (matching infeasible): extra rounds; gaps are padded
    # with dump-vertex dsts below, so only dst-distinctness matters here
    # Overflow edges go through a compact side buffer: PE one-hot gather
    # copies h'[src] rows (block-local) into dense slots, then scatter calls
    # with fully-packed distinct dsts (no dump padding -> no contended rows).
    # ov_src[slot] = source token, ov_dst per sweep must be dst-distinct.
    overflow.sort()
    ov_sweeps = []       # list of lists of (v, d), dst-distinct each
    while overflow:
        taken, rest, seen_d = [], [], set()
        for v, d in overflow:
            if d in seen_d:
                rest.append((v, d))
            else:
                seen_d.add(d)
                taken.append((v, d))
        taken.sort()                     # by source token => block-grouped
        ov_sweeps.append(taken)
        overflow = rest

    # Pack every round into calls of <= MAX_CALL indices (SWDGE ring cap).
    # Slot gaps inside a round are pointed at dump vertices (V..VP-1, never
    # read back); only trailing pad uses -1.
    MAX_CALL = 4096
    dump = np.arange(V, VP, dtype=np.int64)
    rounds = []          # (col16_offset, slot_off, nk_pad, nk_valid)
    blocks = []
    col16 = 0
    for dsts in round_dsts:
        nk = len(dsts)
        dsts = dsts.copy()
        gap = dsts < 0
        ngap = int(gap.sum())
        if ngap:
            dsts[gap] = dump[np.arange(ngap) % len(dump)]
        c0 = 0
        while c0 < nk:
            nkc = min(MAX_CALL, nk - c0)
            nkc_pad = -(-nkc // 128) * 128
            blk16 = np.full((16, nkc_pad // 16), -1, dtype=np.int16)
            i = np.arange(nkc)
            blk16[i % 16, i // 16] = dsts[c0:c0 + nkc].astype(np.int16)
            blocks.append(np.tile(blk16, (8, 1)))
            rounds.append((col16, c0, nkc_pad, nkc))
            col16 += nkc_pad // 16
            c0 += nkc

    # overflow: dense slot list (sources via PE one-hot gather into ov_buf)
    ov_src = []          # source token per dense slot
    ov_rounds = []       # (col16_offset, slot_off_in_ovbuf, nk_pad, nk)
    slot = 0
    for taken in ov_sweeps:
        nk = len(taken)
        dsts_arr = np.array([d for _, d in taken], dtype=np.int16)
        ov_src.extend(v for v, _ in taken)
        nk_pad = -(-nk // 128) * 128
        ov_src.extend([0] * (nk_pad - nk))          # pad slots: copy of row 0
        blk16 = np.full((16, nk_pad // 16), -1, dtype=np.int16)
        i = np.arange(nk)
        blk16[i % 16, i // 16] = dsts_arr
        blocks.append(np.tile(blk16, (8, 1)))
        ov_rounds.append((col16, slot, nk_pad, nk))
        col16 += nk_pad // 16
        slot += nk_pad
    ov_src = np.array(ov_src, dtype=np.int64)
    NOV = len(ov_src)                                # total ov_buf slots
    # one-hot gather stationaries: per psum-tile of 128 dense slots, one
    # [128,128] stationary per source block touched (zero cols elsewhere);
    # psum accumulation merges the groups
    ov_mms = []          # (tile64_idx, (si, nsub), src_block, onehot [128,64])
    for t0 in range(0, NOV, 64):
        seg = ov_src[t0:t0 + 64]
        subs = []
        j = 0
        while j < len(seg):
            sb = int(seg[j]) // 128
            j2 = j
            while j2 < len(seg) and int(seg[j2]) // 128 == sb:
                j2 += 1
            oh = np.zeros((128, 64), dtype=np.float32)
            oh[seg[j:j2] % 128, np.arange(j, j2)] = 1.0
            subs.append((sb, oh))
            j = j2
        for si, (sb, oh) in enumerate(subs):
            ov_mms.append((t0 // 64, (si, len(subs)), sb, oh))
    idx_all = (np.concatenate(blocks, axis=1) if blocks
               else np.zeros((128, 16), dtype=np.int16))
    ov_oh = (np.concatenate([mm[3] for mm in ov_mms], axis=1)
             if ov_mms else np.zeros((128, 1), dtype=np.float32))

    dinv_r = np.zeros(VP, dtype=np.float32)
    dinv_r[:V] = dinv[order]

    xyzT = np.zeros((3, VP), dtype=np.float32)
    xyzT[:, :V] = vertex_xyz[order].T

    # input layer: stationary [k=3, fb, m=128]
    win = np.empty((3, FB, 128), dtype=np.float32)
    for fb in range(FB):
        win[:, fb, :] = input_W[fb * 128:(fb + 1) * 128, :3].T
    # per-core bias c_i = latent_i @ Wlat.T + b_in  -> [128, FB]
    c = latent @ input_W[:, 3:].T + input_b           # [B, 256]
    cbs = [np.ascontiguousarray(c[i].reshape(FB, 128).T) for i in range(B)]

    # conv weights as moving tensors [k=128, l, kb, n=256]
    wt = np.empty((128, NL, FB, HID), dtype=np.float32)
    for l in range(NL):
        for kb in range(FB):
            wt[:, l, kb, :] = conv_W[l][:, kb * 128:(kb + 1) * 128].T
    # output layer moving [k=128, kb, 3]
    wout = np.empty((128, FB, 3), dtype=np.float32)
    for kb in range(FB):
        wout[:, kb, :] = out_W[:, kb * 128:(kb + 1) * 128].T

    bias_pp = np.empty((128, NL, FB), dtype=np.float32)
    for l in range(NL):
        bias_pp[:, l, :] = conv_b[l].reshape(FB, 128).T

    dinv_tok = np.ascontiguousarray(dinv_r.reshape(NB, 128).T)      # [128, NB]
    diag = np.zeros((128, NB, 128), dtype=np.float32)
    p = np.arange(128)
    for r in range(NB):
        diag[p, r, p] = dinv_r[r * 128 + p]

    dev = dict(
        ov_oh=ov_oh.astype(BF16),
        xyzT=xyzT.astype(BF16),
        win=win.astype(BF16),
        wt=wt.astype(BF16),
        wout=wout.astype(BF16),
        bias_pp=bias_pp,
        dinv_tok=dinv_tok,
        diag=diag.astype(BF16),
        idx_all=idx_all,
    )
    meta = dict(rounds=rounds, idx_w=idx_all.shape[1], order=order,
                out_b=out_b, cbs=cbs, ov_rounds=ov_rounds,
                ov_mms=[m[:3] for m in ov_mms], nov=NOV,
                ohw=ov_oh.shape[1])
    return dev, meta


def build(meta):
    from concourse import bacc, tile, mybir
    from concourse import tile_utils
    tile_utils.max_sbuf_usage = 206 * 1024

    rounds = meta['rounds']
    idx_w = meta['idx_w']
    ov_rounds = meta['ov_rounds']
    ov_mms = meta['ov_mms']
    nov = meta['nov']
    ohw = meta['ohw']
    dt = mybir.dt
    alu = mybir.AluOpType
    act_fn = mybir.ActivationFunctionType

    nc = bacc.Bacc(None, target_bir_lowering=False, debug=False)

    d_xyzT = nc.declare_dram_parameter('xyzT', [3, VP], dt.bfloat16, isOutput=False)
    d_win = nc.declare_dram_parameter('win', [3, FB, 128], dt.bfloat16, isOutput=False)
    d_cb = nc.declare_dram_parameter('cb', [128, FB], dt.float32, isOutput=False)
    d_wt = nc.declare_dram_parameter('wt', [128, NL, FB, HID], dt.bfloat16, isOutput=False)
    d_wout = nc.declare_dram_parameter('wout', [128, FB, 3], dt.bfloat16, isOutput=False)
    d_bias = nc.declare_dram_parameter('bias_pp', [128, NL, FB], dt.float32, isOutput=False)
    d_dinv = nc.declare_dram_parameter('dinv_tok', [128, NB], dt.float32, isOutput=False)
    d_diag = nc.declare_dram_parameter('diag', [128, NB, 128], dt.bfloat16, isOutput=False)
    d_idx = nc.declare_dram_parameter('idx_all', [128, idx_w], dt.int16, isOutput=False)
    d_ovoh = nc.declare_dram_parameter('ov_oh', [128, ohw], dt.bfloat16, isOutput=False)
    d_out = nc.declare_dram_parameter('out', [128, NB, 3], dt.float32, isOutput=True)

    with tile.TileContext(nc) as tc:
        with (
            tc.tile_pool(name='const', bufs=1) as const,
            tc.tile_pool(name='work', bufs=1) as work,
            tc.tile_pool(name='pp', bufs=6, space='PSUM') as ppool,
        ):
            # constants (xyzT shares its slot with the overflow buffer:
            # xyzT is only read in the input layer, before ovb's first write)
            t_xyzT = work.tile([3, VP], dt.bfloat16, tag='ovb')
            t_win = const.tile([3, FB, 128], dt.bfloat16)
            t_cb = const.tile([128, FB], dt.float32)
            t_wt = const.tile([128, NL, FB, HID], dt.bfloat16)
            t_wout = const.tile([128, FB, 3], dt.bfloat16)
            t_bias = const.tile([128, NL, FB], dt.float32)
            t_dinv = const.tile([128, NB], dt.float32)
            t_diag = const.tile([128, NB, 128], dt.bfloat16)
            t_idx = const.tile([128, idx_w], dt.int16)
            t_ovoh = const.tile([128, ohw], dt.bfloat16)
            nc.sync.dma_start(t_ovoh[:], d_ovoh[:])
            for t, d in ((t_xyzT, d_xyzT), (t_win, d_win), (t_cb, d_cb),
                         (t_wt, d_wt), (t_wout, d_wout), (t_bias, d_bias),
                         (t_dinv, d_dinv), (t_diag, d_diag), (t_idx, d_idx)):
                nc.sync.dma_start(t[:], d[:])

            # working state
            xT = work.tile([128, FB, VP], dt.bfloat16)     # layer input, [f, v]
            hp = work.tile([128, NB, HID], dt.bfloat16)    # h' token layout
            sE = work.tile([128, 40, HID], dt.bfloat16)    # agg, even ranks
            sO = work.tile([128, 40, HID], dt.bfloat16)    # agg, odd ranks
            ovb = None
            if nov:
                ovb = work.tile([128, nov // 128, HID], dt.bfloat16, tag='ovb')
                assert nov // 128 * HID * 2 <= VP * 2

            # ---- input layer: x0 = relu(xyz @ Wxyz^T + c_b) ----
            for fb in range(FB):
                c0 = 0
                while c0 < VP:
                    w = min(MM_CHUNK, VP - c0)
                    pin = ppool.tile([128, MM_CHUNK], dt.float32, tag='ps')
                    nc.tensor.matmul(pin[:, :w], t_win[:, fb, :], t_xyzT[:, c0:c0 + w],
                                     start=True, stop=True)
                    nc.scalar.activation(xT[:, fb, c0:c0 + w], pin[:, :w],
                                         act_fn.Relu, bias=t_cb[:, fb:fb + 1], scale=1.0)
                    c0 += w

            # ---- GCN layers ----
            for l in range(NL):
                # h' = dinv * (x @ W_l^T), token layout
                for vb in range(NB):
                    pu = ppool.tile([128, HID], dt.float32, tag='ps')
                    for kb in range(FB):
                        nc.tensor.matmul(pu[:], xT[:, kb, vb * 128:(vb + 1) * 128],
                                         t_wt[:, l, kb, :],
                                         start=(kb == 0), stop=(kb == FB - 1))
                    if vb % 2 == 0:
                        nc.vector.tensor_scalar(hp[:, vb, :], pu[:],
                                                t_dinv[:, vb:vb + 1], None, alu.mult)
                    else:
                        nc.scalar.mul(hp[:, vb, :], pu[:], t_dinv[:, vb:vb + 1])

                # aggregation: init with self-loops, then scatter rounds
                nc.vector.tensor_copy(sE[:, 0:40, :], hp[:, 0:NB:2, :])
                nc.vector.tensor_copy(sO[:, 0:39, :], hp[:, 1:NB:2, :])
                for (col16, slot_off, nk_pad, nk) in rounds:
                    s0 = slot_off // 128
                    nc.gpsimd.dma_scatter_add(
                        sE[:],
                        hp[:, s0:s0 + nk_pad // 128, :],
                        t_idx[:, col16:col16 + nk_pad // 16],
                        nk_pad, nk, HID,
                        parity_reg=0,
                        out_ap_other=sO[:],
                        sbuf_tokens_per_rank=128,
                    )
                # overflow edges: compact via PE one-hot gather, then scatter
                if nov:
                    ohc = 0
                    pt_ov = None
                    for (tile_i, (si, nsub), sb) in ov_mms:
                        if si == 0:
                            pt_ov = ppool.tile([64, HID], dt.float32, tag='ps')
                        nc.tensor.matmul(pt_ov[:],
                                         t_ovoh[:, ohc:ohc + 64],
                                         hp[:, sb, :],
                                         start=(si == 0), stop=(si == nsub - 1))
                        ohc += 64
                        if si == nsub - 1:
                            po2 = (tile_i % 2) * 64
                            nc.vector.tensor_copy(
                                ovb[po2:po2 + 64, tile_i // 2, :], pt_ov[:])
                    for (col16, slot_off, nk_pad, nk) in ov_rounds:
                        s0 = slot_off // 128
                        nc.gpsimd.dma_scatter_add(
                            sE[:],
                            ovb[:, s0:s0 + nk_pad // 128, :],
                            t_idx[:, col16:col16 + nk_pad // 16],
                            nk_pad, nk, HID,
                            parity_reg=0,
                            out_ap_other=sO[:],
                            sbuf_tokens_per_rank=128,
                        )

                # x_{l+1} = relu(dinv * s + b): transpose via diag(dinv) matmul
                for fb in range(FB):
                    for vq in range(0, NB, 4):
                        nvb = min(4, NB - vq)
                        pt = ppool.tile([128, 4, 128], dt.float32, tag='ps')
                        for j in range(nvb):
                            vb = vq + j
                            stile = sE if vb % 2 == 0 else sO
                            g = vb // 2
                            nc.tensor.matmul(pt[:, j, :],
                                             stile[:, g, fb * 128:(fb + 1) * 128],
                                             t_diag[:, vb, :],
                                             start=True, stop=True)
                        dstap = xT[:, fb, vq * 128:vq * 128 + nvb * 128]
                        srcap = pt[:, 0:nvb, :]
                        if vq % 8 == 0:
                            nc.vector.tensor_scalar(dstap, srcap,
                                                    t_bias[:, l, fb:fb + 1], 0.0,
                                                    alu.add, alu.max)
                        else:
                            nc.scalar.activation(dstap, srcap, act_fn.Relu,
                                                 bias=t_bias[:, l, fb:fb + 1], scale=1.0)

            # ---- output layer: disp = x5 @ out_W^T (out_b added on host) ----
            po = ppool.tile([128, NB, 3], dt.float32, tag='ps')
            for vb in range(NB):
                for kb in range(FB):
                    nc.tensor.matmul(po[:, vb, :],
                                     xT[:, kb, vb * 128:(vb + 1) * 128],
                                     t_wout[:, kb, :],
                                     start=(kb == 0), stop=(kb == FB - 1))
            outsb = work.tile([128, NB, 3], dt.float32)
            nc.scalar.copy(outsb[:], po[:])
            nc.sync.dma_start(d_out[:], outsb[:])

    nc.compile()
    return nc


_CACHE = {}


def _get_compiled(meta):
    key = tuple(meta['rounds'])
    if key not in _CACHE:
        _CACHE[key] = build(meta)
    return _CACHE[key]


def run(inputs, trace=False, trace_kwargs=None):
    """Run the kernel. Returns (output [B, V, 3] f32, BassKernelResults)."""
    _install_axon_ntff_shim()
    from concourse.bass_utils import run_bass_kernel_spmd

    dev, meta = preprocess(inputs)
    nc = _get_compiled(meta)

    in_maps = []
    for i in range(B):
        m = dict(dev)
        m['cb'] = meta['cbs'][i]
        in_maps.append(m)

    kw = {}
    if trace:
        kw = dict(trace=True, trace_kwargs=trace_kwargs or {})
    res = run_bass_kernel_spmd(nc, in_maps, core_ids=list(range(B)), **kw)

    order = meta['order']
    out_b = meta['out_b']
    out = np.empty((B, V, 3), dtype=np.float32)
    for i in range(B):
        o = np.asarray(res.results[i]['out'])          # [128, NB, 3]
        flat = o.transpose(1, 0, 2).reshape(VP, 3)     # v' = r*128 + p
        out[i, order, :] = flat[:V]
    out += out_b
    return out, res


def kernel(**inputs) -> np.ndarray:
    out, _ = run(inputs, trace=False)
    return out
